# revision 7
# baseline (speedup 1.0000x reference)
"""Trainium2 Bass kernel for nn_DiscriminativeLoss_86242943304305.

The reference loss is einsum('bfl,blk->', pred, one_hot(target)) with
target values always in [0, 16) == the one-hot bin count, so the mask
term sums to exactly 1.0 at every pixel and the loss equals
prediction.sum().  The kernel is therefore a pure memory-bound global
sum of the [16, 8, 512, 512] f32 prediction tensor; `target` never
needs to be read.

Sharding: data-parallel over the batch axis — core i reduces batches
[2i, 2i+2) (16 MiB each); the host sums the per-core partials (the
"all-reduce" of the sharding hint, done host-side since the output is
one scalar).

Implementation: raw Bacc (no TileContext — its kernel-tail drain +
double all-engine barrier costs ~13 us at this kernel's ~50 us scale).
Per core, 16 tiles of [128, 2048] f32 (1 MiB) stream in on the sync
HWDGE ring with 4 buffers per consumer pool; even tiles are reduced on
the vector engine (reduce_sum), odd tiles on the scalar engine
(activation Identity with accum_out), so either engine only has to
cover half the 103 Gelem/s the DMA delivers.  The [128, 16] partial
block DMAs out and the host does the final sum in fp64.
"""

import numpy as np

_N_CORES = 8
_B, _F, _H, _W = 16, 8, 512, 512
_ELEMS_PER_CORE = (_B // _N_CORES) * _F * _H * _W  # 4,194,304
_P = 128
_TILE_M = 4096
_NTILES = _ELEMS_PER_CORE // (_P * _TILE_M)  # 8
_HALF = _NTILES // 2  # tiles per consumer engine
_NBUF = 3  # buffers per consumer pool

_cached_nc = None


def _emit(nc, x, out):
    """Emit the raw-bacc program. x: DRAM [NTILES, P, TILE_M] f32,
    out: DRAM [P, NTILES] f32 (col k < HALF: vector partial of tile 2k;
    col HALF+k: scalar partial of tile 2k+1)."""
    import contextlib

    import concourse.mybir as mybir

    with contextlib.ExitStack() as st:
        slot_v = [
            st.enter_context(
                nc.sbuf_tensor(f"slot_v{s}", [_P, _TILE_M], mybir.dt.float32)
            )
            for s in range(_NBUF)
        ]
        slot_a = [
            st.enter_context(
                nc.sbuf_tensor(f"slot_a{s}", [_P, _TILE_M], mybir.dt.float32)
            )
            for s in range(_NBUF)
        ]
        acc = st.enter_context(
            nc.sbuf_tensor("acc", [_P, _NTILES], mybir.dt.float32)
        )
        sem_lv = [
            st.enter_context(nc.semaphore(name=f"sem_lv{s}")) for s in range(_NBUF)
        ]
        sem_la = [
            st.enter_context(nc.semaphore(name=f"sem_la{s}")) for s in range(_NBUF)
        ]
        sem_v = st.enter_context(nc.semaphore(name="sem_v"))
        sem_a = st.enter_context(nc.semaphore(name="sem_a"))
        sem_out = st.enter_context(nc.semaphore(name="sem_out"))
        blk = st.enter_context(nc.Block(no_gpsimd_drain=True))

        @blk.sync
        def _(sync):
            for k in range(_HALF):
                s = k % _NBUF
                if k >= _NBUF:
                    sync.wait_ge(sem_v, k - _NBUF + 1)
                sync.dma_start(slot_v[s][:], x[2 * k]).then_inc(sem_lv[s], 16)
            sync.wait_ge(sem_v, _HALF)
            sync.wait_ge(sem_a, _HALF)
            sync.dma_start(out[:], acc[:]).then_inc(sem_out, 16)
            sync.wait_ge(sem_out, 16)

        @blk.gpsimd
        def _(gpsimd):
            for k in range(_HALF):
                s = k % _NBUF
                if k >= _NBUF:
                    gpsimd.wait_ge(sem_a, k - _NBUF + 1)
                gpsimd.dma_start(slot_a[s][:], x[2 * k + 1]).then_inc(sem_la[s], 16)

        @blk.vector
        def _(vector):
            for k in range(_HALF):
                s, r = k % _NBUF, k // _NBUF
                vector.wait_ge(sem_lv[s], 16 * (r + 1))
                nc.vector.reduce_sum(
                    acc[:, k : k + 1], slot_v[s][:], axis=mybir.AxisListType.X
                ).then_inc(sem_v, 1)

        @blk.scalar
        def _(scalar):
            for k in range(_HALF):
                s, r = k % _NBUF, k // _NBUF
                scalar.wait_ge(sem_la[s], 16 * (r + 1))
                nc.scalar.activation(
                    slot_a[s][:],
                    slot_a[s][:],
                    mybir.ActivationFunctionType.Identity,
                    accum_out=acc[:, _HALF + k : _HALF + k + 1],
                ).then_inc(sem_a, 1)


def _build():
    global _cached_nc
    if _cached_nc is not None:
        return _cached_nc

    import concourse.bacc as bacc
    import concourse.mybir as mybir

    nc = bacc.Bacc(
        "TRN2", target_bir_lowering=False, debug=False, num_devices=_N_CORES
    )
    x = nc.dram_tensor(
        "x", [_NTILES, _P, _TILE_M], mybir.dt.float32, kind="ExternalInput"
    )
    out = nc.dram_tensor("out", [_P, _NTILES], mybir.dt.float32, kind="ExternalOutput")
    _emit(nc, x, out)
    nc.compile()
    _cached_nc = nc
    return nc


def kernel(prediction: np.ndarray, target: np.ndarray) -> np.ndarray:
    from concourse.bass_utils import run_bass_kernel_spmd

    pred = np.ascontiguousarray(prediction, dtype=np.float32).reshape(
        _N_CORES, _NTILES, _P, _TILE_M
    )
    in_maps = [{"x": pred[i]} for i in range(_N_CORES)]
    nc = _build()
    res = run_bass_kernel_spmd(nc, in_maps, core_ids=list(range(_N_CORES)))
    partials = np.stack([r["out"] for r in res.results])
    total = partials.astype(np.float64).sum()
    return np.array(total, dtype=np.float32)


# revision 8
# speedup vs baseline: 1.0403x; 1.0403x over previous
"""Trainium2 Bass kernel for nn_DiscriminativeLoss_86242943304305.

The reference loss is einsum('bfl,blk->', pred, one_hot(target)) with
target values always in [0, 16) == the one-hot bin count, so the mask
term sums to exactly 1.0 at every pixel and the loss equals
prediction.sum().  The kernel is therefore a pure memory-bound global
sum of the [16, 8, 512, 512] f32 prediction tensor; `target` never
needs to be read.

Sharding: data-parallel over the batch axis — core i reduces batches
[2i, 2i+2) (16 MiB each); the host sums the per-core partials (the
"all-reduce" of the sharding hint, done host-side since the output is
one scalar).

Implementation: raw Bacc (no TileContext — its kernel-tail drain +
double all-engine barrier costs ~13 us at this kernel's ~50 us scale).
Per core, 16 tiles of [128, 2048] f32 (1 MiB) stream in on the sync
HWDGE ring with 4 buffers per consumer pool; even tiles are reduced on
the vector engine (reduce_sum), odd tiles on the scalar engine
(activation Identity with accum_out), so either engine only has to
cover half the 103 Gelem/s the DMA delivers.  The [128, 16] partial
block DMAs out and the host does the final sum in fp64.
"""

import numpy as np

_N_CORES = 8
_B, _F, _H, _W = 16, 8, 512, 512
_ELEMS_PER_CORE = (_B // _N_CORES) * _F * _H * _W  # 4,194,304
_P = 128
_TILE_M = 4096
_NTILES = _ELEMS_PER_CORE // (_P * _TILE_M)  # 8
_HALF = _NTILES // 2  # tiles per consumer engine
_NBUF = 3  # buffers per consumer pool

_cached_nc = None


def _emit(nc, x, out):
    """Emit the raw-bacc program. x: DRAM [NTILES, P, TILE_M] f32,
    out: DRAM [P, NTILES] f32 (col k < HALF: vector partial of tile 2k;
    col HALF+k: scalar partial of tile 2k+1)."""
    import contextlib

    import concourse.mybir as mybir

    with contextlib.ExitStack() as st:
        slot_v = [
            st.enter_context(
                nc.sbuf_tensor(f"slot_v{s}", [_P, _TILE_M], mybir.dt.float32)
            )
            for s in range(_NBUF)
        ]
        slot_a = [
            st.enter_context(
                nc.sbuf_tensor(f"slot_a{s}", [_P, _TILE_M], mybir.dt.float32)
            )
            for s in range(_NBUF)
        ]
        acc = st.enter_context(
            nc.sbuf_tensor("acc", [_P, _NTILES], mybir.dt.float32)
        )
        sem_lv = [
            st.enter_context(nc.semaphore(name=f"sem_lv{s}")) for s in range(_NBUF)
        ]
        sem_la = [
            st.enter_context(nc.semaphore(name=f"sem_la{s}")) for s in range(_NBUF)
        ]
        sem_v = st.enter_context(nc.semaphore(name="sem_v"))
        sem_a = st.enter_context(nc.semaphore(name="sem_a"))
        sem_out = st.enter_context(nc.semaphore(name="sem_out"))
        # Engine streams are emitted directly (no nc.Block()) — the Block
        # exit all-engine barrier costs ~4 us at this kernel's scale.  Each
        # engine's stream self-terminates only after its own work is done
        # (consumers retire their last op; sync waits out the store DMA).
        for i in range(_NTILES):
            k, s = i // 2, (i // 2) % _NBUF
            if i % 2 == 0:
                if k >= _NBUF:
                    nc.sync.wait_ge(sem_v, k - _NBUF + 1)
                nc.sync.dma_start(slot_v[s][:], x[i]).then_inc(sem_lv[s], 16)
            else:
                if k >= _NBUF:
                    nc.sync.wait_ge(sem_a, k - _NBUF + 1)
                nc.sync.dma_start(slot_a[s][:], x[i]).then_inc(sem_la[s], 16)
        nc.sync.wait_ge(sem_v, _HALF)
        nc.sync.wait_ge(sem_a, _HALF)
        nc.sync.dma_start(out[:], acc[:]).then_inc(sem_out, 16)
        nc.sync.wait_ge(sem_out, 16)

        for k in range(_HALF):
            s, r = k % _NBUF, k // _NBUF
            nc.vector.wait_ge(sem_lv[s], 16 * (r + 1))
            nc.vector.reduce_sum(
                acc[:, k : k + 1], slot_v[s][:], axis=mybir.AxisListType.X
            ).then_inc(sem_v, 1)

        for k in range(_HALF):
            s, r = k % _NBUF, k // _NBUF
            nc.scalar.wait_ge(sem_la[s], 16 * (r + 1))
            nc.scalar.activation(
                slot_a[s][:],
                slot_a[s][:],
                mybir.ActivationFunctionType.Identity,
                accum_out=acc[:, _HALF + k : _HALF + k + 1],
            ).then_inc(sem_a, 1)


def _build():
    global _cached_nc
    if _cached_nc is not None:
        return _cached_nc

    import concourse.bacc as bacc
    import concourse.mybir as mybir

    nc = bacc.Bacc(
        "TRN2", target_bir_lowering=False, debug=False, num_devices=_N_CORES
    )
    x = nc.dram_tensor(
        "x", [_NTILES, _P, _TILE_M], mybir.dt.float32, kind="ExternalInput"
    )
    out = nc.dram_tensor("out", [_P, _NTILES], mybir.dt.float32, kind="ExternalOutput")
    _emit(nc, x, out)
    nc.compile()
    _cached_nc = nc
    return nc


def kernel(prediction: np.ndarray, target: np.ndarray) -> np.ndarray:
    from concourse.bass_utils import run_bass_kernel_spmd

    pred = np.ascontiguousarray(prediction, dtype=np.float32).reshape(
        _N_CORES, _NTILES, _P, _TILE_M
    )
    in_maps = [{"x": pred[i]} for i in range(_N_CORES)]
    nc = _build()
    res = run_bass_kernel_spmd(nc, in_maps, core_ids=list(range(_N_CORES)))
    partials = np.stack([r["out"] for r in res.results])
    total = partials.astype(np.float64).sum()
    return np.array(total, dtype=np.float32)


# revision 9
# speedup vs baseline: 1.1989x; 1.1525x over previous
"""Trainium2 Bass kernel for nn_DiscriminativeLoss_86242943304305.

The reference loss is einsum('bfl,blk->', pred, one_hot(target)) with
target values always in [0, 16) == the one-hot bin count, so the mask
term sums to exactly 1.0 at every pixel and the loss equals
prediction.sum().  The kernel is therefore a pure memory-bound global
sum of the [16, 8, 512, 512] f32 prediction tensor; `target` never
needs to be read.

Sharding: data-parallel over the batch axis — core i reduces batches
[2i, 2i+2) (16 MiB each); the host sums the per-core partials (the
"all-reduce" of the sharding hint, done host-side since the output is
one scalar).

Implementation: raw Bacc (no TileContext — its kernel-tail drain +
double all-engine barrier costs ~13 us at this kernel's ~50 us scale).
Per core, 16 tiles of [128, 2048] f32 (1 MiB) stream in on the sync
HWDGE ring with 4 buffers per consumer pool; even tiles are reduced on
the vector engine (reduce_sum), odd tiles on the scalar engine
(activation Identity with accum_out), so either engine only has to
cover half the 103 Gelem/s the DMA delivers.  The [128, 16] partial
block DMAs out and the host does the final sum in fp64.
"""

import numpy as np

_N_CORES = 8
_B, _F, _H, _W = 16, 8, 512, 512
_ELEMS_PER_CORE = (_B // _N_CORES) * _F * _H * _W  # 4,194,304
_P = 128
_TILE_M = 4096
_NTILES = _ELEMS_PER_CORE // (_P * _TILE_M)  # 8
_HALF = _NTILES // 2  # tiles per consumer engine
_NBUF = 4  # buffers per consumer pool (== _HALF: every tile gets its own slot)

_cached_nc = None


def _emit(nc, x, out):
    """Emit the raw-bacc program. x: DRAM [NTILES, P, TILE_M] f32,
    out: DRAM [P, NTILES] f32 (col k < HALF: vector partial of tile 2k;
    col HALF+k: scalar partial of tile 2k+1)."""
    import contextlib

    import concourse.mybir as mybir

    with contextlib.ExitStack() as st:
        slot_v = [
            st.enter_context(
                nc.sbuf_tensor(f"slot_v{s}", [_P, _TILE_M], mybir.dt.float32)
            )
            for s in range(_NBUF)
        ]
        slot_a = [
            st.enter_context(
                nc.sbuf_tensor(f"slot_a{s}", [_P, _TILE_M], mybir.dt.float32)
            )
            for s in range(_NBUF)
        ]
        acc = st.enter_context(
            nc.sbuf_tensor("acc", [_P, _NTILES], mybir.dt.float32)
        )
        sem_lv = [
            st.enter_context(nc.semaphore(name=f"sem_lv{s}")) for s in range(_NBUF)
        ]
        sem_la = [
            st.enter_context(nc.semaphore(name=f"sem_la{s}")) for s in range(_NBUF)
        ]
        sem_v = st.enter_context(nc.semaphore(name="sem_v"))
        sem_a = st.enter_context(nc.semaphore(name="sem_a"))
        sem_out = st.enter_context(nc.semaphore(name="sem_out"))
        # Engine streams are emitted directly (no nc.Block()) — the Block
        # exit all-engine barrier costs ~4 us at this kernel's scale.  Each
        # engine's stream self-terminates only after its own work is done
        # (consumers retire their last op; sync waits out the store DMA).
        for i in range(_NTILES):
            k, s = i // 2, (i // 2) % _NBUF
            if i % 2 == 0:
                if k >= _NBUF:
                    nc.sync.wait_ge(sem_v, k - _NBUF + 1)
                nc.sync.dma_start(slot_v[s][:], x[i]).then_inc(sem_lv[s], 16)
            else:
                if k >= _NBUF:
                    nc.sync.wait_ge(sem_a, k - _NBUF + 1)
                nc.sync.dma_start(slot_a[s][:], x[i]).then_inc(sem_la[s], 16)
        nc.sync.wait_ge(sem_v, _HALF)
        nc.sync.wait_ge(sem_a, _HALF)
        nc.sync.dma_start(out[:], acc[:]).then_inc(sem_out, 16)
        nc.sync.wait_ge(sem_out, 16)

        for k in range(_HALF):
            s, r = k % _NBUF, k // _NBUF
            nc.vector.wait_ge(sem_lv[s], 16 * (r + 1))
            nc.vector.reduce_sum(
                acc[:, k : k + 1], slot_v[s][:], axis=mybir.AxisListType.X
            ).then_inc(sem_v, 1)

        for k in range(_HALF):
            s, r = k % _NBUF, k // _NBUF
            nc.scalar.wait_ge(sem_la[s], 16 * (r + 1))
            nc.scalar.activation(
                slot_a[s][:],
                slot_a[s][:],
                mybir.ActivationFunctionType.Identity,
                accum_out=acc[:, _HALF + k : _HALF + k + 1],
            ).then_inc(sem_a, 1)


def _build():
    global _cached_nc
    if _cached_nc is not None:
        return _cached_nc

    import concourse.bacc as bacc
    import concourse.mybir as mybir

    nc = bacc.Bacc(
        "TRN2", target_bir_lowering=False, debug=False, num_devices=_N_CORES
    )
    x = nc.dram_tensor(
        "x", [_NTILES, _P, _TILE_M], mybir.dt.float32, kind="ExternalInput"
    )
    out = nc.dram_tensor("out", [_P, _NTILES], mybir.dt.float32, kind="ExternalOutput")
    _emit(nc, x, out)
    nc.compile()
    _cached_nc = nc
    return nc


def kernel(prediction: np.ndarray, target: np.ndarray) -> np.ndarray:
    from concourse.bass_utils import run_bass_kernel_spmd

    pred = np.ascontiguousarray(prediction, dtype=np.float32).reshape(
        _N_CORES, _NTILES, _P, _TILE_M
    )
    in_maps = [{"x": pred[i]} for i in range(_N_CORES)]
    nc = _build()
    res = run_bass_kernel_spmd(nc, in_maps, core_ids=list(range(_N_CORES)))
    partials = np.stack([r["out"] for r in res.results])
    total = partials.astype(np.float64).sum()
    return np.array(total, dtype=np.float32)


# revision 10
# speedup vs baseline: 1.2177x; 1.0157x over previous
"""Trainium2 Bass kernel for nn_DiscriminativeLoss_86242943304305.

The reference loss is einsum('bfl,blk->', pred, one_hot(target)) with
target values always in [0, 16) == the one-hot bin count, so the mask
term sums to exactly 1.0 at every pixel and the loss equals
prediction.sum().  The kernel is therefore a pure memory-bound global
sum of the [16, 8, 512, 512] f32 prediction tensor; `target` never
needs to be read.

Sharding: data-parallel over the batch axis — core i reduces batches
[2i, 2i+2) (16 MiB each); the host sums the per-core partials (the
"all-reduce" of the sharding hint, done host-side since the output is
one scalar).

Implementation: raw Bacc (no TileContext — its kernel-tail drain +
double all-engine barrier costs ~13 us at this kernel's ~50 us scale).
Per core, 16 tiles of [128, 2048] f32 (1 MiB) stream in on the sync
HWDGE ring with 4 buffers per consumer pool; even tiles are reduced on
the vector engine (reduce_sum), odd tiles on the scalar engine
(activation Identity with accum_out), so either engine only has to
cover half the 103 Gelem/s the DMA delivers.  The [128, 16] partial
block DMAs out and the host does the final sum in fp64.
"""

import numpy as np

_N_CORES = 8
_B, _F, _H, _W = 16, 8, 512, 512
_ELEMS_PER_CORE = (_B // _N_CORES) * _F * _H * _W  # 4,194,304
_P = 128
_TILE_M = 4096
_NTILES = _ELEMS_PER_CORE // (_P * _TILE_M)  # 8
_HALF = _NTILES // 2  # tiles per consumer engine
_NBUF = 4  # buffers per consumer pool (== _HALF: every tile gets its own slot)

_cached_nc = None


def _emit(nc, x, out):
    """Emit the raw-bacc program. x: DRAM [NTILES, P, TILE_M] f32,
    out: DRAM [P, NTILES] f32 (col k < HALF: vector partial of tile 2k;
    col HALF+k: scalar partial of tile 2k+1)."""
    import contextlib

    import concourse.mybir as mybir

    with contextlib.ExitStack() as st:
        slot_v = [
            st.enter_context(
                nc.sbuf_tensor(f"slot_v{s}", [_P, _TILE_M], mybir.dt.float32)
            )
            for s in range(_NBUF)
        ]
        slot_a = [
            st.enter_context(
                nc.sbuf_tensor(f"slot_a{s}", [_P, _TILE_M], mybir.dt.float32)
            )
            for s in range(_NBUF)
        ]
        acc = st.enter_context(
            nc.sbuf_tensor("acc", [_P, _NTILES], mybir.dt.float32)
        )
        sem_lv = [
            st.enter_context(nc.semaphore(name=f"sem_lv{s}")) for s in range(_NBUF)
        ]
        sem_la = [
            st.enter_context(nc.semaphore(name=f"sem_la{s}")) for s in range(_NBUF)
        ]
        sem_v = st.enter_context(nc.semaphore(name="sem_v"))
        sem_a = st.enter_context(nc.semaphore(name="sem_a"))
        sem_out = st.enter_context(nc.semaphore(name="sem_out"))
        # Engine streams are emitted directly (no nc.Block()) — the Block
        # exit all-engine barrier costs ~4 us at this kernel's scale.  Each
        # engine's stream self-terminates only after its own work is done
        # (consumers retire their last op; sync waits out the store DMA).
        for i in range(_NTILES):
            k, s = i // 2, (i // 2) % _NBUF
            if i % 2 == 0:
                if k >= _NBUF:
                    nc.sync.wait_ge(sem_v, k - _NBUF + 1)
                nc.sync.dma_start(slot_v[s][:], x[i]).then_inc(sem_lv[s], 16)
            else:
                if k >= _NBUF:
                    nc.sync.wait_ge(sem_a, k - _NBUF + 1)
                nc.sync.dma_start(slot_a[s][:], x[i]).then_inc(sem_la[s], 16)
        nc.sync.wait_ge(sem_v, _HALF)
        nc.sync.wait_ge(sem_a, _HALF)
        nc.sync.dma_start(out[:], acc[:]).then_inc(sem_out, 16)
        nc.sync.wait_ge(sem_out, 16)

        for k in range(_HALF):
            s, r = k % _NBUF, k // _NBUF
            nc.vector.wait_ge(sem_lv[s], 16 * (r + 1))
            nc.vector.reduce_sum(
                acc[:, k : k + 1], slot_v[s][:], axis=mybir.AxisListType.X
            ).then_inc(sem_v, 1)

        for k in range(_HALF):
            s, r = k % _NBUF, k // _NBUF
            nc.scalar.wait_ge(sem_la[s], 16 * (r + 1))
            nc.scalar.activation(
                slot_a[s][:],
                slot_a[s][:],
                mybir.ActivationFunctionType.Identity,
                accum_out=acc[:, _HALF + k : _HALF + k + 1],
            ).then_inc(sem_a, 1)


def _build():
    global _cached_nc
    if _cached_nc is not None:
        return _cached_nc

    import concourse.bacc as bacc
    import concourse.mybir as mybir

    nc = bacc.Bacc(
        "TRN2", target_bir_lowering=False, debug=False, num_devices=_N_CORES
    )
    x = nc.dram_tensor(
        "x", [_NTILES, _P, _TILE_M], mybir.dt.float32, kind="ExternalInput"
    )
    out = nc.dram_tensor("out", [_P, _NTILES], mybir.dt.float32, kind="ExternalOutput")
    _emit(nc, x, out)
    nc.compile()
    _strip_startup_barrier(nc)
    _cached_nc = nc
    return nc


def _strip_startup_barrier(nc):
    """Remove the Bass preamble all-engine barrier (~3 us of engine
    boot-skew absorption).  Every cross-engine dependency in this kernel
    is ordered by explicit load/consumer semaphores, so the barrier only
    delays the first DMA dispatch."""

    def _is_barrier_inst(i):
        if i.name.startswith("barrier_"):
            return True
        if i.opcode == "Drain" and i.sync_info is not None:
            refs = [w.ant_name for w in i.sync_info.on_wait] + [
                getattr(u, "ant_name", "") for u in i.sync_info.on_update
            ]
            return any(r and r.startswith("barrier_") for r in refs)
        return False

    for fn in nc.m.functions:
        for blk in fn.blocks:
            doomed = [i for i in blk.instructions if _is_barrier_inst(i)]
            for i in doomed:
                blk.instructions.remove(i)


def kernel(prediction: np.ndarray, target: np.ndarray) -> np.ndarray:
    from concourse.bass_utils import run_bass_kernel_spmd

    pred = np.ascontiguousarray(prediction, dtype=np.float32).reshape(
        _N_CORES, _NTILES, _P, _TILE_M
    )
    in_maps = [{"x": pred[i]} for i in range(_N_CORES)]
    nc = _build()
    res = run_bass_kernel_spmd(nc, in_maps, core_ids=list(range(_N_CORES)))
    partials = np.stack([r["out"] for r in res.results])
    total = partials.astype(np.float64).sum()
    return np.array(total, dtype=np.float32)
